# revision 15
# baseline (speedup 1.0000x reference)
"""Trainium2 Bass kernel for nn_DifferentiableAttention_9818295239474.

Sharding: data-parallel over B (4) x tensor-parallel over head groups (2) = 8 cores.
Core i handles batch b=i//2, head-group g=i%2 (q-heads [16g,16g+16), kv-heads [4g,4g+4),
x rows [512g, 512g+512)). Each core produces a full [1024,1024] partial out2 and the
host sums the two partials per batch (proj2 distributes over proj1's partial sums, so
no on-device collective is needed).

Key trick: the reference's "raw reshape" head split means q-head h is
reshape(Qfull[b, 32h:32h+32, :2048], (1024, 64)) (and kv-head j the analogous
(128-row, 512-col) block), i.e. the sequence index s = 32a + r interleaves an x-row
index a (free axis after the QKV matmul) with a W-column index r (partition axis).
All attention is done in a per-tile scrambled s-order; order only has to be globally
consistent, and proj2's contraction over s is order-invariant once W2's rows are
host-permuted to match.
"""
import math
import numpy as np

B, S, H = 4, 1024, 1024
DEPTH = 1
LAMBDA_INIT = 0.8 - 0.6 * math.exp(-0.3 * DEPTH)

F32R = True  # bitcast fp32->float32r on big-matmul SBUF operands (full rate at N>=256)

_CACHE = {}


def _build_masks():
    """Causal masks for diagonal subs: masks[d2*2+vb][m', f] in {0,1}.

    s_k - 512c = 256*d2 + 2*m' + vb;  f = 256v + 16u + a with
    s_q - 512c = 32a + 2u + v."""
    d2 = np.arange(2)[:, None, None, None]
    mp = np.arange(128)[None, None, :, None]
    vb = np.arange(2)[None, :, None, None]
    f = np.arange(512)[None, None, None, :]
    v, u, a = 1 - f // 256, (f % 256) // 16, f % 16
    sq = 32 * a + 2 * u + v
    sk = 256 * d2 + 2 * mp + vb
    return np.ascontiguousarray((sq >= sk).astype(np.float32))  # [2,2,128,512]


def _sq_order():
    idx = np.arange(1024)
    c, rr = idx // 256, idx % 256
    cc, v = c // 2, 1 - c % 2
    u, a = rr // 16, rr % 16
    return 32 * (16 * cc + a) + 2 * u + v


def _build_program():
    import concourse.bacc as bacc
    import concourse.mybir as mybir
    import concourse.tile as tile

    dt = mybir.dt
    f32, f32r, bf16 = dt.float32, dt.float32r, dt.bfloat16
    AF = mybir.ActivationFunctionType
    ALU = mybir.AluOpType

    fr = f32r if F32R else f32

    def r(ap):
        return ap

    nc = bacc.Bacc("TRN2", target_bir_lowering=False, debug=False,
                   enable_asserts=False, num_devices=8)

    xT_d = nc.dram_tensor("xT", [1024, 512], bf16, kind="ExternalInput").ap()
    wqkv_d = nc.dram_tensor("wqkv", [1024, 3072], bf16, kind="ExternalInput").ap()
    w1_d = nc.dram_tensor("w1", [512, 1024], bf16, kind="ExternalInput").ap()
    w2_d = nc.dram_tensor("w2", [1024, 1024], bf16, kind="ExternalInput").ap()
    masks_d = nc.dram_tensor("masks", [4 * 128, 512], bf16, kind="ExternalInput").ap()
    ident_d = nc.dram_tensor("ident", [128, 64], bf16, kind="ExternalInput").ap()
    cvec_d = nc.dram_tensor("cvec", [128, 128], fr, kind="ExternalInput").ap()
    out_d = nc.dram_tensor("out", [1024, 1024], f32, kind="ExternalOutput").ap()

    with tile.TileContext(nc) as tc:
        with tc.tile_pool(name="persist", bufs=1) as persist, \
             tc.tile_pool(name="wstream", bufs=3) as wstream, \
             tc.tile_pool(name="ps_big", bufs=2, space="PSUM") as ps_big, \
             tc.tile_pool(name="ps_st", bufs=2, space="PSUM") as ps_st, \
             tc.tile_pool(name="ps_u", bufs=2, space="PSUM") as ps_u, \
             tc.tile_pool(name="ppool", bufs=6) as ppool, \
             tc.tile_pool(name="dpool", bufs=3) as dpool, \
             tc.tile_pool(name="tpool", bufs=2) as tpool, \
             tc.tile_pool(name="opool", bufs=4) as opool:

            xT = persist.tile([128, 8 * 512], bf16, tag="xT")
            qt = persist.tile([128, 16 * 512], bf16, tag="qt")
            kt = persist.tile([128, 4 * 512], bf16, tag="kt")
            kts = persist.tile([128, 4 * 512], bf16, tag="kts")
            vt = persist.tile([128, 4 * 512], bf16, tag="vt")
            vsb = persist.tile([128, 32 * 65], bf16, tag="vsb")
            msk = persist.tile([128, 4 * 512], bf16, tag="msk")
            ident = persist.tile([128, 64], bf16, tag="ident")
            cvec = persist.tile([128, 128], fr, tag="cvec")
            at = persist.tile([128, 4 * 1024], bf16, tag="at")
            atsc = persist.tile([128, 1024], bf16, tag="atsc")
            w1 = persist.tile([128, 4 * 1024], bf16, tag="w1")
            w2 = persist.tile([128, 8 * 1024], bf16, tag="w2")
            yt = persist.tile([128, 8 * 1024], bf16, tag="yt")

            sync = nc.sync

            sync.dma_start(out=xT.rearrange("p (t s) -> p t s", t=8),
                           in_=xT_d.rearrange("(t p) s -> p t s", p=128))

            # ---------------- QKV^T projection ----------------
            for t in range(24):
                wtile = wstream.tile([128, 8 * 128], bf16, tag="wtile")
                sync.dma_start(
                    out=wtile.rearrange("p (ht c) -> p ht c", ht=8),
                    in_=wqkv_d[:, 128 * t:128 * (t + 1)].rearrange(
                        "(ht p) c -> p ht c", p=128))
                ps = ps_big.tile([128, 512], f32, tag="big")
                for ht in range(8):
                    nc.tensor.matmul(
                        ps[:], r(wtile[:, 128 * ht:128 * (ht + 1)]),
                        r(xT[:, 512 * ht:512 * (ht + 1)]),
                        start=(ht == 0), stop=(ht == 7))
                if t < 16:
                    # qt layout: free = 512*h + 256*cc + 16*u + a' (u = t)
                    dst = qt.rearrange("p (h cc ap) -> p h cc ap", h=16, cc=2)
                    dst = dst[:, :, :, 16 * t:16 * t + 16]
                    srcv = ps.rearrange("p (h cc ap) -> p h cc ap", h=16, cc=2)
                    nc.vector.tensor_copy(dst, srcv)
                else:
                    # kt/vt layout: free = 512*j + 64*tk + 4*ab' + ub (ub = t-16 or t-20)
                    kv = kt if t < 20 else vt
                    ub = t - 16 if t < 20 else t - 20
                    dst = kv.rearrange("p (j tk ab four) -> p j tk ab four",
                                       j=4, tk=8, ab=16)
                    dst = dst[:, :, :, :, ub:ub + 1].squeeze(4)
                    srcv = ps.rearrange("p (j tk ab) -> p j tk ab", j=4, tk=8)
                    nc.vector.tensor_copy(dst, srcv)

            # Deferred input DMAs: needed only from the attention phase on
            # (keeps them out of the wqkv stream's way at kernel start).
            sync.dma_start(out=msk.rearrange("p (t s) -> p t s", t=4),
                           in_=masks_d.rearrange("(t p) s -> p t s", p=128))
            sync.dma_start(out=ident[:], in_=ident_d)
            sync.dma_start(out=cvec[:], in_=cvec_d)
            sync.dma_start(out=w1.rearrange("p (t s) -> p t s", t=4),
                           in_=w1_d.rearrange("(t p) s -> p t s", p=128))
            sync.dma_start(out=w2.rearrange("p (t s) -> p t s", t=8),
                           in_=w2_d.rearrange("(t p) s -> p t s", p=128))

            # KT_swap: swapped partition halves (SBUF->SBUF DMA)
            sync.dma_start(out=kts[0:64, :], in_=kt[64:128, :])
            sync.dma_start(out=kts[64:128, :], in_=kt[0:64, :])

            # ---------------- V tiles via PE transpose ----------------
            # block bidx = j*8 + tk2*2 + vb: V[s_k = 256*tk2 + 2*m' + vb, d]
            for j in range(4):
                for tk2 in range(4):
                    for vb in range(2):
                        base = (8 * j + 2 * tk2 + vb) * 65
                        ps = ps_big.tile([128, 64], bf16, tag="big")
                        nc.tensor.transpose(
                            ps[:, :],
                            vt[64 * vb:64 * vb + 64,
                               512 * j + 128 * tk2:512 * j + 128 * (tk2 + 1)],
                            ident[64 * vb:64 * vb + 64, :])
                        nc.vector.tensor_copy(vsb[:, base:base + 64], ps[:, :])
                        nc.vector.memset(vsb[:, base + 64:base + 65], 1.0)

            # ---------------- attention ----------------
            for p in range(8):
                j = p // 2
                for half in range(2):
                    h = 2 * p + half
                    for c in range(2):
                        subs = [(tk2, vb) for tk2 in range(2 * (c + 1))
                                for vb in range(2)]
                        up = ps_u.tile([128, 512], f32, tag="u")
                        for sidx, (tk2, vb) in enumerate(subs):
                            # v=1 (lhsT base-partition 64) must write PSUM
                            # offset 0 — the base64+offset combo faults on HW.
                            pt = ppool.tile([128, 512], bf16, tag="p")
                            st = ps_st.tile([128, 512], f32, tag="st")
                            for v in range(2):
                                ksrc = kt if vb == v else kts
                                nc.tensor.matmul(
                                    st[:, 256 * (1 - v):256 * (1 - v) + 256],
                                    ksrc[64 * v:64 * v + 64,
                                         512 * j + 128 * tk2:512 * j + 128 * (tk2 + 1)],
                                    qt[64 * v:64 * v + 64,
                                       512 * h + 256 * c:512 * h + 256 * c + 256],
                                    start=True, stop=True)
                                nc.scalar.activation(
                                    pt[:, 256 * (1 - v):256 * (1 - v) + 256],
                                    st[:, 256 * (1 - v):256 * (1 - v) + 256],
                                    AF.Exp, scale=0.125)
                            d2 = tk2 - 2 * c
                            if 0 <= d2 < 2:
                                nc.vector.tensor_mul(
                                    pt[:], pt[:],
                                    msk[:, (d2 * 2 + vb) * 512:(d2 * 2 + vb + 1) * 512])
                            bidx = 8 * j + 2 * tk2 + vb
                            nc.tensor.matmul(
                                up[0:65, :],
                                vsb[:, bidx * 65:bidx * 65 + 65], pt[:],
                                start=(sidx == 0), stop=(sidx == len(subs) - 1))
                        usb = dpool.tile([128, 512], fr, tag="usb")
                        nc.any.tensor_copy(usb[0:65, :], up[0:65, :])
                        inv = dpool.tile([128, 512], fr, tag="inv")
                        with nc.allow_low_precision(reason="f32r inv"):
                            nc.vector.reciprocal(inv[64:65, :], usb[64:65, :])
                        db = ps_big.tile([128, 512], f32, tag="big")
                        nc.tensor.matmul(
                            db[0:64, :],
                            r(cvec[64:65, 64 * half:64 * half + 64]),
                            r(inv[64:65, :]), start=True, stop=True)
                        dst = at if p % 2 == 0 else atsc
                        col = (1024 * (p // 2) if p % 2 == 0 else 0) + 512 * c
                        if half == 0:
                            nc.vector.tensor_mul(
                                dst[0:64, col:col + 512], usb[0:64, :],
                                db[0:64, :])
                        else:
                            t1 = tpool.tile([128, 512], f32, tag="t1")
                            nc.vector.tensor_mul(
                                t1[0:64, :], usb[0:64, :], db[0:64, :])
                            nc.vector.tensor_add(
                                dst[0:64, col:col + 512],
                                dst[0:64, col:col + 512], t1[0:64, :])
                if p % 2 == 1:
                    sync.dma_start(
                        out=at[64:128, 1024 * (p // 2):1024 * (p // 2 + 1)],
                        in_=atsc[0:64, :])

            # ---------------- proj1: y = A^T.T @ W1 ----------------
            for st_i in range(8):
                for nh in range(2):
                    ps = ps_big.tile([128, 512], f32, tag="big")
                    for k in range(4):
                        nc.tensor.matmul(
                            ps[:],
                            r(at[:, 1024 * k + 128 * st_i:1024 * k + 128 * (st_i + 1)]),
                            r(w1[:, 1024 * k + 512 * nh:1024 * k + 512 * (nh + 1)]),
                            start=(k == 0), stop=(k == 3))
                    nc.vector.tensor_copy(
                        yt[:, 1024 * st_i + 512 * nh:1024 * st_i + 512 * (nh + 1)],
                        ps[:])

            # ---------------- proj2: out2 = y^T @ W2 ----------------
            for it in range(8):
                for nh in range(2):
                    ps = ps_big.tile([128, 512], f32, tag="big")
                    for st_i in range(8):
                        nc.tensor.matmul(
                            ps[:],
                            r(yt[:, 1024 * st_i + 128 * it:1024 * st_i + 128 * (it + 1)]),
                            r(w2[:, 1024 * st_i + 512 * nh:1024 * st_i + 512 * (nh + 1)]),
                            start=(st_i == 0), stop=(st_i == 7))
                    ob = opool.tile([128, 512], f32, tag="ob")
                    nc.vector.tensor_copy(ob[:], ps[:])
                    sync.dma_start(
                        out=out_d[128 * it:128 * (it + 1), 512 * nh:512 * (nh + 1)],
                        in_=ob[:])

    nc.compile()
    return nc


def make_in_maps(x, Wqkv, Wproj, lam):
    import ml_dtypes
    c0 = 1.0 - LAMBDA_INIT
    masks = _build_masks().astype(ml_dtypes.bfloat16).reshape(4 * 128, 512)
    ident = np.ascontiguousarray(np.tile(np.eye(64), (2, 1))).astype(ml_dtypes.bfloat16)
    cvec = np.zeros((128, 128), np.float32)
    cvec[64, 0:64] = c0
    cvec[64, 64:128] = -c0 * lam
    order = _sq_order()
    bf = ml_dtypes.bfloat16
    w2 = np.ascontiguousarray(Wproj[order, :]).astype(bf)
    wqkv_bf = Wqkv.astype(bf)
    in_maps = []
    for core in range(8):
        b, g = core // 2, core % 2
        in_maps.append({
            "xT": np.ascontiguousarray(x[b, 512 * g:512 * (g + 1), :].T).astype(bf),
            "wqkv": wqkv_bf,
            "w1": np.ascontiguousarray(Wproj[512 * g:512 * (g + 1), :]).astype(bf),
            "w2": w2,
            "masks": masks,
            "ident": ident,
            "cvec": cvec,
        })
    return in_maps


def kernel(x, Wqkv, Wproj, lambda_q1, lambda_k1, lambda_q2, lambda_k2, _trace=False):
    from concourse.bass_utils import run_bass_kernel_spmd

    x = np.asarray(x, np.float32)
    Wqkv = np.asarray(Wqkv, np.float32)
    Wproj = np.asarray(Wproj, np.float32)
    lam1 = float(np.exp(np.sum(np.asarray(lambda_q1, np.float64)
                               * np.asarray(lambda_k1, np.float64))))
    lam2 = float(np.exp(np.sum(np.asarray(lambda_q2, np.float64)
                               * np.asarray(lambda_k2, np.float64))))
    lam = lam1 - lam2 + LAMBDA_INIT

    if "nc" not in _CACHE:
        _CACHE["nc"] = _build_program()
    nc = _CACHE["nc"]

    in_maps = make_in_maps(x, Wqkv, Wproj, lam)
    res = run_bass_kernel_spmd(nc, in_maps, core_ids=list(range(8)), trace=_trace)
    _CACHE["last_results"] = res
    out = np.zeros((B, S, H), np.float32)
    for core in range(8):
        out[core // 2] += res.results[core]["out"]
    return out



# revision 16
# speedup vs baseline: 1.2364x; 1.2364x over previous
"""Trainium2 Bass kernel for nn_DifferentiableAttention_9818295239474.

Sharding: data-parallel over B (4) x tensor-parallel over head groups (2) = 8 cores.
Core i handles batch b=i//2, head-group g=i%2 (q-heads [16g,16g+16), kv-heads [4g,4g+4),
x rows [512g, 512g+512)). Each core produces a full [1024,1024] partial out2 and the
host sums the two partials per batch (proj2 distributes over proj1's partial sums, so
no on-device collective is needed).

Key trick: the reference's "raw reshape" head split means q-head h is
reshape(Qfull[b, 32h:32h+32, :2048], (1024, 64)) (and kv-head j the analogous
(128-row, 512-col) block), i.e. the sequence index s = 32a + r interleaves an x-row
index a (free axis after the QKV matmul) with a W-column index r (partition axis).
All attention is done in a per-tile scrambled s-order; order only has to be globally
consistent, and proj2's contraction over s is order-invariant once W2's rows are
host-permuted to match.
"""
import math
import numpy as np

B, S, H = 4, 1024, 1024
DEPTH = 1
LAMBDA_INIT = 0.8 - 0.6 * math.exp(-0.3 * DEPTH)

F32R = True  # bitcast fp32->float32r on big-matmul SBUF operands (full rate at N>=256)

_CACHE = {}


def _build_masks():
    """Causal masks for diagonal subs: masks[d2*2+vb][m', f] in {0,1}.

    s_k - 512c = 256*d2 + 2*m' + vb;  f = 256v + 16u + a with
    s_q - 512c = 32a + 2u + v."""
    d2 = np.arange(2)[:, None, None, None]
    mp = np.arange(128)[None, None, :, None]
    vb = np.arange(2)[None, :, None, None]
    f = np.arange(512)[None, None, None, :]
    v, u, a = f // 256, (f % 256) // 16, f % 16
    sq = 32 * a + 2 * u + v
    sk = 256 * d2 + 2 * mp + vb
    return np.ascontiguousarray((sq >= sk).astype(np.float32))  # [2,2,128,512]


def _sq_order():
    idx = np.arange(1024)
    c, rr = idx // 256, idx % 256
    cc, v = c // 2, c % 2
    u, a = rr // 16, rr % 16
    return 32 * (16 * cc + a) + 2 * u + v


def _build_program():
    import concourse.bacc as bacc
    import concourse.mybir as mybir
    import concourse.tile as tile

    dt = mybir.dt
    f32, f32r, bf16 = dt.float32, dt.float32r, dt.bfloat16
    AF = mybir.ActivationFunctionType
    ALU = mybir.AluOpType

    fr = f32r if F32R else f32

    def r(ap):
        return ap

    nc = bacc.Bacc("TRN2", target_bir_lowering=False, debug=False,
                   enable_asserts=False, num_devices=8)

    xT_d = nc.dram_tensor("xT", [1024, 512], bf16, kind="ExternalInput").ap()
    wqkv_d = nc.dram_tensor("wqkv", [1024, 3072], bf16, kind="ExternalInput").ap()
    w1_d = nc.dram_tensor("w1", [512, 1024], bf16, kind="ExternalInput").ap()
    w2_d = nc.dram_tensor("w2", [1024, 1024], bf16, kind="ExternalInput").ap()
    masks_d = nc.dram_tensor("masks", [4 * 128, 512], bf16, kind="ExternalInput").ap()
    ident_d = nc.dram_tensor("ident", [128, 64], bf16, kind="ExternalInput").ap()
    cvec_d = nc.dram_tensor("cvec", [128, 128], fr, kind="ExternalInput").ap()
    out_d = nc.dram_tensor("out", [1024, 1024], f32, kind="ExternalOutput").ap()

    with tile.TileContext(nc) as tc:
        with tc.tile_pool(name="persist", bufs=1) as persist, \
             tc.tile_pool(name="wstream", bufs=3) as wstream, \
             tc.tile_pool(name="ps_big", bufs=2, space="PSUM") as ps_big, \
             tc.tile_pool(name="ps_st", bufs=3, space="PSUM") as ps_st, \
             tc.tile_pool(name="ps_u", bufs=2, space="PSUM") as ps_u, \
             tc.tile_pool(name="ppool", bufs=6) as ppool, \
             tc.tile_pool(name="dpool", bufs=3) as dpool, \
             tc.tile_pool(name="tpool", bufs=2) as tpool, \
             tc.tile_pool(name="opool", bufs=4) as opool:

            xT = persist.tile([128, 8 * 512], bf16, tag="xT")
            qt = persist.tile([128, 16 * 512], bf16, tag="qt")
            qts = persist.tile([128, 16 * 512], bf16, tag="qts")
            kt = persist.tile([128, 4 * 512], bf16, tag="kt")
            kts = persist.tile([128, 4 * 512], bf16, tag="kts")
            vt = persist.tile([128, 4 * 512], bf16, tag="vt")
            vsb = persist.tile([128, 32 * 65], bf16, tag="vsb")
            msk = persist.tile([128, 4 * 512], bf16, tag="msk")
            ident = persist.tile([128, 64], bf16, tag="ident")
            cvec = persist.tile([128, 128], fr, tag="cvec")
            at = persist.tile([128, 4 * 1024], bf16, tag="at")
            atsc = persist.tile([128, 1024], bf16, tag="atsc")
            w1 = persist.tile([128, 4 * 1024], bf16, tag="w1")
            w2 = persist.tile([128, 8 * 1024], bf16, tag="w2")
            yt = persist.tile([128, 8 * 1024], bf16, tag="yt")

            sync = nc.sync

            sync.dma_start(out=xT.rearrange("p (t s) -> p t s", t=8),
                           in_=xT_d.rearrange("(t p) s -> p t s", p=128))

            # ---------------- QKV^T projection ----------------
            for t in range(24):
                wtile = wstream.tile([128, 8 * 128], bf16, tag="wtile")
                sync.dma_start(
                    out=wtile.rearrange("p (ht c) -> p ht c", ht=8),
                    in_=wqkv_d[:, 128 * t:128 * (t + 1)].rearrange(
                        "(ht p) c -> p ht c", p=128))
                ps = ps_big.tile([128, 512], f32, tag="big")
                for ht in range(8):
                    nc.tensor.matmul(
                        ps[:], r(wtile[:, 128 * ht:128 * (ht + 1)]),
                        r(xT[:, 512 * ht:512 * (ht + 1)]),
                        start=(ht == 0), stop=(ht == 7))
                if t < 16:
                    # qt layout: free = 512*h + 256*cc + 16*u + a' (u = t)
                    dst = qt.rearrange("p (h cc ap) -> p h cc ap", h=16, cc=2)
                    dst = dst[:, :, :, 16 * t:16 * t + 16]
                    srcv = ps.rearrange("p (h cc ap) -> p h cc ap", h=16, cc=2)
                    nc.vector.tensor_copy(dst, srcv)
                else:
                    # kt/vt layout: free = 512*j + 64*tk + 4*ab' + ub (ub = t-16 or t-20)
                    kv = kt if t < 20 else vt
                    ub = t - 16 if t < 20 else t - 20
                    dst = kv.rearrange("p (j tk ab four) -> p j tk ab four",
                                       j=4, tk=8, ab=16)
                    dst = dst[:, :, :, :, ub:ub + 1].squeeze(4)
                    srcv = ps.rearrange("p (j tk ab) -> p j tk ab", j=4, tk=8)
                    nc.vector.tensor_copy(dst, srcv)

            # Deferred input DMAs: needed only from the attention phase on
            # (keeps them out of the wqkv stream's way at kernel start).
            sync.dma_start(out=msk.rearrange("p (t s) -> p t s", t=4),
                           in_=masks_d.rearrange("(t p) s -> p t s", p=128))
            sync.dma_start(out=ident[:], in_=ident_d)
            sync.dma_start(out=cvec[:], in_=cvec_d)
            sync.dma_start(out=w1.rearrange("p (t s) -> p t s", t=4),
                           in_=w1_d.rearrange("(t p) s -> p t s", p=128))
            sync.dma_start(out=w2.rearrange("p (t s) -> p t s", t=8),
                           in_=w2_d.rearrange("(t p) s -> p t s", p=128))

            # KT_swap: swapped partition halves (SBUF->SBUF DMA)
            sync.dma_start(out=kts[0:64, :], in_=kt[64:128, :])
            sync.dma_start(out=kts[64:128, :], in_=kt[0:64, :])
            # QT_swap: v=1 parity at partitions 0:64 so all score matmuls
            # use base-0 operands (base-64 lhsT + merged ACT faults on HW)
            sync.dma_start(out=qts[0:64, :], in_=qt[64:128, :])

            # ---------------- V tiles via PE transpose ----------------
            # block bidx = j*8 + tk2*2 + vb: V[s_k = 256*tk2 + 2*m' + vb, d]
            for j in range(4):
                for tk2 in range(4):
                    for vb in range(2):
                        base = (8 * j + 2 * tk2 + vb) * 65
                        ps = ps_big.tile([128, 64], bf16, tag="big")
                        nc.tensor.transpose(
                            ps[:, :],
                            vt[64 * vb:64 * vb + 64,
                               512 * j + 128 * tk2:512 * j + 128 * (tk2 + 1)],
                            ident[64 * vb:64 * vb + 64, :])
                        nc.vector.tensor_copy(vsb[:, base:base + 64], ps[:, :])
                        nc.vector.memset(vsb[:, base + 64:base + 65], 1.0)

            # ---------------- attention ----------------
            for p in range(8):
                j = p // 2
                for half in range(2):
                    h = 2 * p + half
                    for c in range(2):
                        subs = [(tk2, vb) for tk2 in range(2 * (c + 1))
                                for vb in range(2)]
                        up = ps_u.tile([128, 512], f32, tag="u")
                        for sidx, (tk2, vb) in enumerate(subs):
                            pt = ppool.tile([128, 512], bf16, tag="p")
                            st = ps_st.tile([128, 512], f32, tag="st")
                            for v in range(2):
                                # v=1 slices live at partitions 0:64 of the
                                # swapped copies (all-base-0 operands).
                                if v == 0:
                                    ksrc = kt if vb == 0 else kts
                                    qsrc = qt
                                else:
                                    ksrc = kts if vb == 1 else kt
                                    qsrc = qts
                                nc.tensor.matmul(
                                    st[:, 256 * v:256 * v + 256],
                                    ksrc[0:64,
                                         512 * j + 128 * tk2:512 * j + 128 * (tk2 + 1)],
                                    qsrc[0:64,
                                         512 * h + 256 * c:512 * h + 256 * c + 256],
                                    start=True, stop=True)
                            nc.scalar.activation(pt[:], st[:], AF.Exp, scale=0.125)
                            d2 = tk2 - 2 * c
                            if 0 <= d2 < 2:
                                nc.vector.tensor_mul(
                                    pt[:], pt[:],
                                    msk[:, (d2 * 2 + vb) * 512:(d2 * 2 + vb + 1) * 512])
                            bidx = 8 * j + 2 * tk2 + vb
                            nc.tensor.matmul(
                                up[0:65, :],
                                vsb[:, bidx * 65:bidx * 65 + 65], pt[:],
                                start=(sidx == 0), stop=(sidx == len(subs) - 1))
                        usb = dpool.tile([128, 512], fr, tag="usb")
                        nc.any.tensor_copy(usb[0:65, :], up[0:65, :])
                        inv = dpool.tile([128, 512], fr, tag="inv")
                        with nc.allow_low_precision(reason="f32r inv"):
                            nc.vector.reciprocal(inv[64:65, :], usb[64:65, :])
                        db = ps_big.tile([128, 512], f32, tag="big")
                        nc.tensor.matmul(
                            db[0:64, :],
                            r(cvec[64:65, 64 * half:64 * half + 64]),
                            r(inv[64:65, :]), start=True, stop=True)
                        dst = at if p % 2 == 0 else atsc
                        col = (1024 * (p // 2) if p % 2 == 0 else 0) + 512 * c
                        if half == 0:
                            nc.vector.tensor_mul(
                                dst[0:64, col:col + 512], usb[0:64, :],
                                db[0:64, :])
                        else:
                            t1 = tpool.tile([128, 512], f32, tag="t1")
                            nc.vector.tensor_mul(
                                t1[0:64, :], usb[0:64, :], db[0:64, :])
                            nc.vector.tensor_add(
                                dst[0:64, col:col + 512],
                                dst[0:64, col:col + 512], t1[0:64, :])
                if p % 2 == 1:
                    sync.dma_start(
                        out=at[64:128, 1024 * (p // 2):1024 * (p // 2 + 1)],
                        in_=atsc[0:64, :])

            # ---------------- proj1: y = A^T.T @ W1 ----------------
            for st_i in range(8):
                for nh in range(2):
                    ps = ps_big.tile([128, 512], f32, tag="big")
                    for k in range(4):
                        nc.tensor.matmul(
                            ps[:],
                            r(at[:, 1024 * k + 128 * st_i:1024 * k + 128 * (st_i + 1)]),
                            r(w1[:, 1024 * k + 512 * nh:1024 * k + 512 * (nh + 1)]),
                            start=(k == 0), stop=(k == 3))
                    nc.vector.tensor_copy(
                        yt[:, 1024 * st_i + 512 * nh:1024 * st_i + 512 * (nh + 1)],
                        ps[:])

            # ---------------- proj2: out2 = y^T @ W2 ----------------
            for it in range(8):
                for nh in range(2):
                    ps = ps_big.tile([128, 512], f32, tag="big")
                    for st_i in range(8):
                        nc.tensor.matmul(
                            ps[:],
                            r(yt[:, 1024 * st_i + 128 * it:1024 * st_i + 128 * (it + 1)]),
                            r(w2[:, 1024 * st_i + 512 * nh:1024 * st_i + 512 * (nh + 1)]),
                            start=(st_i == 0), stop=(st_i == 7))
                    ob = opool.tile([128, 512], f32, tag="ob")
                    nc.vector.tensor_copy(ob[:], ps[:])
                    sync.dma_start(
                        out=out_d[128 * it:128 * (it + 1), 512 * nh:512 * (nh + 1)],
                        in_=ob[:])

    nc.compile()
    return nc


def make_in_maps(x, Wqkv, Wproj, lam):
    import ml_dtypes
    c0 = 1.0 - LAMBDA_INIT
    masks = _build_masks().astype(ml_dtypes.bfloat16).reshape(4 * 128, 512)
    ident = np.ascontiguousarray(np.tile(np.eye(64), (2, 1))).astype(ml_dtypes.bfloat16)
    cvec = np.zeros((128, 128), np.float32)
    cvec[64, 0:64] = c0
    cvec[64, 64:128] = -c0 * lam
    order = _sq_order()
    bf = ml_dtypes.bfloat16
    w2 = np.ascontiguousarray(Wproj[order, :]).astype(bf)
    wqkv_bf = Wqkv.astype(bf)
    in_maps = []
    for core in range(8):
        b, g = core // 2, core % 2
        in_maps.append({
            "xT": np.ascontiguousarray(x[b, 512 * g:512 * (g + 1), :].T).astype(bf),
            "wqkv": wqkv_bf,
            "w1": np.ascontiguousarray(Wproj[512 * g:512 * (g + 1), :]).astype(bf),
            "w2": w2,
            "masks": masks,
            "ident": ident,
            "cvec": cvec,
        })
    return in_maps


def kernel(x, Wqkv, Wproj, lambda_q1, lambda_k1, lambda_q2, lambda_k2, _trace=False):
    from concourse.bass_utils import run_bass_kernel_spmd

    x = np.asarray(x, np.float32)
    Wqkv = np.asarray(Wqkv, np.float32)
    Wproj = np.asarray(Wproj, np.float32)
    lam1 = float(np.exp(np.sum(np.asarray(lambda_q1, np.float64)
                               * np.asarray(lambda_k1, np.float64))))
    lam2 = float(np.exp(np.sum(np.asarray(lambda_q2, np.float64)
                               * np.asarray(lambda_k2, np.float64))))
    lam = lam1 - lam2 + LAMBDA_INIT

    if "nc" not in _CACHE:
        _CACHE["nc"] = _build_program()
    nc = _CACHE["nc"]

    in_maps = make_in_maps(x, Wqkv, Wproj, lam)
    res = run_bass_kernel_spmd(nc, in_maps, core_ids=list(range(8)), trace=_trace)
    _CACHE["last_results"] = res
    out = np.zeros((B, S, H), np.float32)
    for core in range(8):
        out[core // 2] += res.results[core]["out"]
    return out

